# revision 1
# baseline (speedup 1.0000x reference)
"""GATNet (3-layer GAT + global mean pool) as a Bass/Tile SPMD kernel on 8 trn2 cores.

Strategy (matches sharding hint, specialized):
 - Edges sorted by dst on host; dst nodes partitioned into 8 contiguous blocks of
   1280 padded nodes (10000 real -> 10240 padded). Core c owns dst block c and all
   edges into it; per-core softmax is fully local (no denominator all-reduce).
 - Node features replicated: every core computes the dense per-node transforms
   xs = x @ W (fp32r matmuls) for all nodes and writes a DRAM node table
   T[n] = [xs(n) | al_src(n) | al_dst(n)]. Attention-logit vectors are algebraically
   folded: al_src = x @ vs with vs[k,h] = sum_c W[k,hC+c] a_s[h,c], same for dst/edge.
 - Per-edge work on each core's edge slice: indirect-DMA gather of T rows by src,
   exp(leaky_relu(al_src+al_dst+al_edge)) per edge, then one-hot segment matmuls
   (M[e,n] = [dst_e == n]) on the PE accumulate both the weighted feature sums and
   the softmax denominators per 128-node group. Softmax normalization is applied
   after aggregation (it commutes with the weighted sum).
 - Self loops handled densely (PyG fill_value='mean' folds to mean of incoming
   al_edge per node). h is transposed on-chip per 128-chunk and AllGathered between
   layers. Final graph mean-pool via one-hot graph matmul + AllReduce.
"""

import os
import sys
import numpy as np

sys.path.insert(0, "/opt/trn_rl_repo")

N, E, G, H = 10000, 160000, 64, 4
NCORES = 8
RANGE = 1280           # padded dst nodes per core
NPAD = NCORES * RANGE  # 10240
NGRP = RANGE // 128    # 10 groups of 128 dst nodes per core
NCHUNK = NPAD // 128   # 80 dense node chunks
DIMS = [(128, 128), (128, 256), (256, 128)]  # (cin, cout) per layer
EDIM = 32
NEG = 0.2

_cache = {}


def _prep(x, edge_index, edge_attr, batch):
    """Host-side sharding/index bookkeeping (numpy, no model flops)."""
    src = np.asarray(edge_index[0], dtype=np.int64)
    dst = np.asarray(edge_index[1], dtype=np.int64)
    order = np.argsort(dst, kind="stable")
    ss, ds = src[order], dst[order]
    eas = np.asarray(edge_attr, dtype=np.float32)[order]
    deg = np.bincount(ds, minlength=N).astype(np.int64)

    bounds = np.searchsorted(ds, np.arange(0, NPAD + 1, 128))
    cnt = bounds[1:] - bounds[:-1]                       # edges per global group
    TPG = int(np.ceil(cnt.max() / 128))                  # tiles per group (uniform)
    NT = NGRP * TPG                                      # tiles per core

    srcidx = np.zeros((NCORES, 128, NT), np.int32)
    dstf = np.full((NCORES, 128, NT), -1.0, np.float32)
    eattrT = np.zeros((NCORES, EDIM, NT * 128), np.float32)
    invdeg = np.ones((NCORES, 128, NGRP), np.float32)
    batchg = np.full((NCORES, 128, NGRP), -1.0, np.float32)

    bat = np.asarray(batch, dtype=np.int64)
    for c in range(NCORES):
        fsrc = np.zeros(NT * 128, np.int64)
        fdst = np.full(NT * 128, -1.0, np.float32)
        fea = np.zeros((NT * 128, EDIM), np.float32)
        for gi in range(NGRP):
            gg = c * NGRP + gi
            lo, hi = bounds[gg], bounds[gg + 1]
            n = hi - lo
            p0 = gi * TPG * 128
            fsrc[p0:p0 + n] = ss[lo:hi]
            fdst[p0:p0 + n] = (ds[lo:hi] - (c * RANGE + gi * 128)).astype(np.float32)
            fea[p0:p0 + n] = eas[lo:hi]
        srcidx[c] = np.ascontiguousarray(fsrc.reshape(NT, 128).T).astype(np.int32)
        dstf[c] = np.ascontiguousarray(fdst.reshape(NT, 128).T)
        eattrT[c] = np.ascontiguousarray(fea.T)
        nodes = c * RANGE + np.arange(RANGE)
        real = nodes < N
        iv = np.ones(RANGE, np.float32)
        iv[real] = 1.0 / np.maximum(deg[nodes[real]], 1)
        invdeg[c] = np.ascontiguousarray(iv.reshape(NGRP, 128).T)
        bg = np.full(RANGE, -1.0, np.float32)
        bg[real] = bat[nodes[real]].astype(np.float32)
        batchg[c] = np.ascontiguousarray(bg.reshape(NGRP, 128).T)

    xT = np.zeros((128, NPAD), np.float32)
    xT[:, :N] = np.asarray(x, dtype=np.float32).T
    invcnt = (1.0 / np.maximum(np.bincount(bat, minlength=G), 1)).astype(np.float32)
    return dict(TPG=TPG, NT=NT, srcidx=srcidx, dstf=dstf, eattrT=eattrT,
                invdeg=invdeg, batchg=batchg, xT=xT, invcnt=invcnt.reshape(G, 1))


def _build(TPG):
    """Trace + compile the SPMD Tile program (identical on all 8 cores)."""
    import concourse.bass as bass
    import concourse.bacc as bacc
    import concourse.tile as tile
    import concourse.mybir as mybir
    from concourse.bass import IndirectOffsetOnAxis
    from concourse.masks import make_identity

    f32 = mybir.dt.float32
    bf16 = mybir.dt.bfloat16
    u16 = mybir.dt.uint16
    i32 = mybir.dt.int32
    Alu = mybir.AluOpType
    Act = mybir.ActivationFunctionType
    NT = NGRP * TPG

    nc = bacc.Bacc("TRN2", target_bir_lowering=False, debug=False, num_devices=NCORES)

    # ---- I/O ----
    t_xT = nc.dram_tensor("xT", [128, NPAD], f32, kind="ExternalInput")
    t_src = nc.dram_tensor("srcidx", [128, NT], i32, kind="ExternalInput")
    t_dst = nc.dram_tensor("dstf", [128, NT], f32, kind="ExternalInput")
    t_eaT = nc.dram_tensor("eattrT", [EDIM, NT * 128], f32, kind="ExternalInput")
    t_ivd = nc.dram_tensor("invdeg", [128, NGRP], f32, kind="ExternalInput")
    t_bg = nc.dram_tensor("batchg", [128, NGRP], f32, kind="ExternalInput")
    t_ivc = nc.dram_tensor("invcnt", [G, 1], f32, kind="ExternalInput")
    t_W, t_We, t_ac, t_b = [], [], [], []
    for l, (cin, cout) in enumerate(DIMS):
        HC = H * cout
        t_W.append(nc.dram_tensor(f"W{l+1}", [cin, HC], f32, kind="ExternalInput"))
        t_We.append(nc.dram_tensor(f"We{l+1}", [EDIM, HC], f32, kind="ExternalInput"))
        t_ac.append(nc.dram_tensor(f"acat{l+1}", [1, 3 * HC], f32, kind="ExternalInput"))
        t_b.append(nc.dram_tensor(f"b{l+1}", [1, cout], f32, kind="ExternalInput"))
    t_out = nc.dram_tensor("out", [G, 128], f32, kind="ExternalOutput")

    with tile.TileContext(nc) as tc:
        # ---- DRAM scratch: node table rows are [xs bf16 (HC) | al_src,al_dst f32 (8)]
        # stored as uint16 rows of HC+16 elements.
        T_a = tc.tile([NPAD, 512 + 16], u16, space="DRAM", name="T_a")[0]
        T_b = tc.tile([NPAD, 1024 + 16], u16, space="DRAM", name="T_b")[0]
        hTs = [tc.tile([DIMS[l][1], RANGE], bf16, space="DRAM", name=f"hTs{l}")[0]
               for l in range(2)]
        dram_sh_ctx = tc.tile_pool(name="dram_sh", bufs=1, space="DRAM")
        dram_sh = dram_sh_ctx.__enter__()
        hTall = [dram_sh.tile([NCORES * DIMS[l][1], RANGE], bf16, name=f"hTall{l}",
                              tag=f"hTall{l}", addr_space="Shared") for l in range(2)]
        pool_in = tc.tile([G, 128], f32, space="DRAM", name="pool_in")[0]
        pool_out = dram_sh.tile([G, 128], f32, name="pool_out", tag="pool_out",
                                addr_space="Shared")

        # ---- resident SBUF ----
        res_ctx = tc.tile_pool(name="res", bufs=1)
        res = res_ctx.__enter__()

        def rt(shape, dtype, nm):
            return res.tile(shape, dtype, name=nm, tag=nm)

        ident = rt([128, 128], bf16, "ident")
        make_identity(nc, ident[:])
        iota_i = rt([128, 128], i32, "iota_i")
        nc.gpsimd.iota(iota_i[:], pattern=[[1, 128]], base=0, channel_multiplier=0)
        iota_f = rt([128, 128], f32, "iota_f")
        nc.vector.tensor_copy(iota_f[:], iota_i[:])
        iota64_i = rt([128, G], i32, "iota64_i")
        nc.gpsimd.iota(iota64_i[:], pattern=[[1, G]], base=0, channel_multiplier=0)
        iota64_f = rt([128, G], f32, "iota64_f")
        nc.vector.tensor_copy(iota64_f[:], iota64_i[:])

        srcidx_s = rt([128, NT], i32, "srcidx_s")
        nc.sync.dma_start(srcidx_s[:], t_src[:])
        dstf_s = rt([128, NT], f32, "dstf_s")
        nc.sync.dma_start(dstf_s[:], t_dst[:])
        ivd_s = rt([128, NGRP], f32, "ivd_s")
        nc.sync.dma_start(ivd_s[:], t_ivd[:])
        bg_s = rt([128, NGRP], f32, "bg_s")
        nc.sync.dma_start(bg_s[:], t_bg[:])
        ivc_s = rt([G, 1], f32, "ivc_s")
        nc.sync.dma_start(ivc_s[:], t_ivc[:])

        alE = rt([128, NT, 12], bf16, "alE")     # per-edge [ve1|ve2|ve3]
        malE = rt([128, NGRP, 12], f32, "malE")  # per-node mean of alE

        pid = nc.gpsimd.partition_id()
        own0 = pid * RANGE  # first own row in T

        def build_M(out_t, gt):
            # M[e, n] = (dst_local[e] == n), 128x128 one-hot (bf16 exact 0/1)
            nc.vector.tensor_tensor(
                out=out_t[:], in0=dstf_s[:, gt:gt + 1].to_broadcast([128, 128]),
                in1=iota_f[:], op=Alu.is_equal)

        # ================= setup: ve vectors, alE rows, mean_alE =================
        with tc.tile_pool(name="setup_sb", bufs=2) as sp, \
             tc.tile_pool(name="setup_ps", bufs=2, space="PSUM") as pp:
            vecat = sp.tile([EDIM, 12], bf16, tag="vecat", bufs=1)
            for l in range(3):
                HC = H * DIMS[l][1]
                We_s = sp.tile([EDIM, HC], f32, tag="wes")
                nc.sync.dma_start(We_s[:], t_We[l][:])
                aeb = sp.tile([EDIM, HC], f32, tag="aeb")
                ae_row = sp.tile([1, HC], f32, tag="aerow")
                nc.sync.dma_start(ae_row[:], t_ac[l][0:1, 2 * HC:3 * HC])
                nc.gpsimd.partition_broadcast(aeb[:], ae_row[:], channels=EDIM)
                tmp = sp.tile([EDIM, HC], f32, tag="etmp")
                nc.vector.tensor_tensor(out=tmp[:], in0=We_s[:], in1=aeb[:], op=Alu.mult)
                vef = sp.tile([EDIM, 4], f32, tag="vef")
                nc.vector.tensor_reduce(
                    out=vef[:], in_=tmp[:].rearrange("p (h c) -> p h c", h=H),
                    axis=mybir.AxisListType.X, op=Alu.add)
                nc.vector.tensor_copy(vecat[:, l * 4:(l + 1) * 4], vef[:])
            # alE rows: per tile t, psum[e,12] = eattrT[:,t]^T @ vecat
            for t in range(NT):
                ea_c = sp.tile([EDIM, 128], bf16, tag="eac")
                nc.gpsimd.dma_start(ea_c[:], t_eaT[:, t * 128:(t + 1) * 128])
                ps = pp.tile([128, 12], f32, tag="psae", space="PSUM")
                nc.tensor.matmul(ps[:], lhsT=ea_c[:], rhs=vecat[:],
                                 start=True, stop=True)
                nc.vector.tensor_copy(alE[:, t, :], ps[:])
            # S_alE per group then mean (self-loop attr = mean of incoming)
            for gi in range(NGRP):
                psS = pp.tile([128, 12], f32, tag="psS", space="PSUM")
                for k in range(TPG):
                    gt = gi * TPG + k
                    Mt = sp.tile([128, 128], bf16, tag="Mt")
                    build_M(Mt, gt)
                    nc.tensor.matmul(psS[:], lhsT=Mt[:], rhs=alE[:, gt, :],
                                     start=(k == 0), stop=(k == TPG - 1),
                                     skip_group_check=True)
                nc.vector.tensor_scalar(out=malE[:, gi, :], in0=psS[:],
                                        scalar1=ivd_s[:, gi:gi + 1], scalar2=None,
                                        op0=Alu.mult)

        # ================= layers =================
        with tc.tile_pool(name="lhs", bufs=1) as lhs_pool:
            for l, (cin, cout) in enumerate(DIMS):
                HC = H * cout
                RW = HC + 16          # u16 elements per T row
                T = T_a if HC == 512 else T_b
                KC = cin // 128

                lhs = [lhs_pool.tile([128, NPAD], bf16, tag=f"lhs{k}",
                                     name=f"lhs{l}_{k}") for k in range(KC)]
                if l == 0:
                    nc.gpsimd.dma_start(lhs[0][:], t_xT[:])  # f32 -> bf16 cast DMA
                else:
                    cprev = DIMS[l - 1][1]
                    for c8 in range(NCORES):
                        for k in range(KC):
                            nc.sync.dma_start(
                                lhs[k][:, c8 * RANGE:(c8 + 1) * RANGE],
                                hTall[l - 1][c8 * cprev + k * 128:
                                             c8 * cprev + (k + 1) * 128, :])

                with tc.tile_pool(name=f"prm{l}", bufs=1) as prm:
                    W_b = [prm.tile([128, HC], bf16, name=f"Wb{l}_{k}") for k in range(KC)]
                    vsvd_b = [prm.tile([128, 8], bf16, name=f"vsvdb{l}_{k}") for k in range(KC)]
                    bb = prm.tile([128, cout], f32, name=f"bb{l}")
                    hT_acc = prm.tile([128, (cout // 128), RANGE], bf16,
                                      name=f"hTacc{l}") if l < 2 else None
                    with tc.tile_pool(name=f"prmt{l}", bufs=1) as prt:
                        W_s = [prt.tile([128, HC], f32, name=f"W{l}_{k}") for k in range(KC)]
                        for k in range(KC):
                            nc.sync.dma_start(W_s[k][:], t_W[l][k * 128:(k + 1) * 128, :])
                            nc.vector.tensor_copy(W_b[k][:], W_s[k][:])
                        ar = prt.tile([1, 3 * HC], f32, name=f"ar{l}")
                        nc.sync.dma_start(ar[:], t_ac[l][:])
                        ab = prt.tile([128, 2 * HC], f32, name=f"ab{l}")
                        nc.gpsimd.partition_broadcast(ab[:], ar[0:1, 0:2 * HC], channels=128)
                        br = prt.tile([1, cout], f32, name=f"br{l}")
                        nc.sync.dma_start(br[:], t_b[l][:])
                        nc.gpsimd.partition_broadcast(bb[:], br[:], channels=128)
                        tmpv = prt.tile([128, HC], f32, name=f"tmpv{l}")
                        vsf = prt.tile([128, 8], f32, name=f"vsf{l}")
                        for k in range(KC):
                            for j, off in ((0, 0), (1, HC)):  # a_src, a_dst
                                nc.vector.tensor_tensor(out=tmpv[:], in0=W_s[k][:],
                                                        in1=ab[:, off:off + HC], op=Alu.mult)
                                nc.vector.tensor_reduce(
                                    out=vsf[:, j * 4:(j + 1) * 4],
                                    in_=tmpv[:].rearrange("p (h c) -> p h c", h=H),
                                    axis=mybir.AxisListType.X, op=Alu.add)
                            nc.vector.tensor_copy(vsvd_b[k][:], vsf[:])

                    # ---- phase 1: node table T = [xs | al_src | al_dst] ----
                    with tc.tile_pool(name=f"p1s{l}", bufs=3) as p1s, \
                         tc.tile_pool(name=f"p1p{l}", bufs=2, space="PSUM") as p1p:
                        for g in range(NCHUNK):
                            ps_xs = p1p.tile([128, HC], f32, tag="psxs", space="PSUM")
                            ps_al = p1p.tile([128, 8], f32, tag="psal", space="PSUM")
                            for k in range(KC):
                                lT = lhs[k][:, g * 128:(g + 1) * 128]
                                for q in range(HC // 512):
                                    nc.tensor.matmul(ps_xs[:, q * 512:(q + 1) * 512],
                                                     lhsT=lT, rhs=W_b[k][:, q * 512:(q + 1) * 512],
                                                     start=(k == 0), stop=(k == KC - 1),
                                                     skip_group_check=True)
                                nc.tensor.matmul(ps_al[:], lhsT=lT, rhs=vsvd_b[k][:],
                                                 start=(k == 0), stop=(k == KC - 1),
                                                 skip_group_check=True)
                            stage = p1s.tile([128, RW], u16, tag="stage")
                            nc.vector.tensor_copy(stage[:, :HC].bitcast(bf16), ps_xs[:])
                            nc.vector.tensor_copy(stage[:, HC:].bitcast(f32), ps_al[:])
                            nc.sync.dma_start(T[g * 128:(g + 1) * 128, :], stage[:])

                    # ---- phase 3: edge aggregation per own 128-node group ----
                    with tc.tile_pool(name=f"p3s{l}", bufs=2) as p3s, \
                         tc.tile_pool(name=f"p3v{l}", bufs=3) as p3v, \
                         tc.tile_pool(name=f"p3p{l}", bufs=2, space="PSUM") as p3p, \
                         tc.tile_pool(name=f"p3q{l}", bufs=2, space="PSUM") as p3q, \
                         tc.tile_pool(name=f"p3r{l}", bufs=(2 if l == 0 else 1), space="PSUM") as p3r, \
                         tc.tile_pool(name=f"p3d{l}", bufs=(2 if l != 1 else 1), space="PSUM") as p3d, \
                         (tc.tile_pool(name=f"p3pool{l}", bufs=1, space="PSUM")
                          if l == 2 else _null()) as plp:
                            ps_pool = plp.tile([G, 128], f32, space="PSUM",
                                               name="ps_pool") if l == 2 else None
                            for gi in range(NGRP):
                                g0 = gi * TPG
                                ownrow = p3s.tile([128, RW], u16, tag="ownrow")
                                nc.gpsimd.dma_start(ownrow[:], T[bass.ds(own0 + gi * 128, 128), :])
                                own_as = ownrow[:, HC:HC + 8].bitcast(f32)      # al_src [128,4]
                                own_ad = ownrow[:, HC + 8:HC + 16].bitcast(f32)  # al_dst [128,4]
                                adG = p3v.tile([128, 4], bf16, tag="adG")
                                nc.vector.tensor_copy(adG[:], own_ad)
                                # gather all TPG tiles of this group in one indirect DMA
                                # NOTE: one indirect DMA per 128-edge tile. Batching all TPG
                                # tiles into a single multi-offset indirect DMA (offset ap
                                # [128,TPG], 3D out) hangs the exec unit on this runtime —
                                # verified twice (strided and contiguous offsets). Keep [128,1].
                                gth = p3s.tile([128, TPG, RW], u16, tag="gth")
                                for k in range(TPG):
                                    nc.gpsimd.indirect_dma_start(
                                        out=gth[:, k, :], out_offset=None, in_=T[:],
                                        in_offset=IndirectOffsetOnAxis(
                                            ap=srcidx_s[:, g0 + k:g0 + k + 1], axis=0))
                                # all M one-hots of the group in one DVE op
                                M_all = p3v.tile([128, TPG, 128], bf16, tag="M_all")
                                dsl = dstf_s[:, g0:g0 + TPG]
                                din = bass.AP(dsl.tensor, dsl.offset,
                                              [dsl.ap[0], dsl.ap[1], [0, 128]])
                                iap = iota_f[:]
                                iin = bass.AP(iap.tensor, iap.offset,
                                              [iap.ap[0], [0, TPG], iap.ap[1]])
                                nc.vector.tensor_tensor(out=M_all[:], in0=din, in1=iin,
                                                        op=Alu.is_equal)
                                ps_agg = p3p.tile([128, HC], f32, tag="psagg", space="PSUM")
                                ps_den = p3d.tile([128, 4], f32, tag="psden", space="PSUM")
                                ps_adeg = p3r.tile([128, TPG * 4], f32, tag="psadeg", space="PSUM")
                                # loop A: expand al_dst to edges via M^T matmuls
                                for k in range(TPG):
                                    ps_mt = p3q.tile([128, 128], bf16, tag="pssm", space="PSUM")
                                    nc.tensor.transpose(ps_mt[:], M_all[:, k, :], ident[:])
                                    mt_s = p3v.tile([128, 128], bf16, tag="mts")
                                    nc.scalar.copy(mt_s[:], ps_mt[:])
                                    nc.tensor.matmul(ps_adeg[:, k * 4:(k + 1) * 4], lhsT=mt_s[:],
                                                     rhs=adG[:], start=True, stop=True,
                                                     skip_group_check=True)
                                # batched z -> ex for the whole group
                                zball = p3v.tile([128, TPG, 4], f32, tag="zball")
                                nc.vector.tensor_tensor(
                                    out=zball[:], in0=gth[:, :, HC:HC + 8].bitcast(f32),
                                    in1=alE[:, g0:g0 + TPG, l * 4:(l + 1) * 4], op=Alu.add)
                                nc.vector.tensor_tensor(out=zball[:], in0=zball[:],
                                                        in1=ps_adeg[:], op=Alu.add)
                                zlall = p3v.tile([128, TPG, 4], f32, tag="zlall")
                                nc.vector.tensor_scalar(out=zlall[:], in0=zball[:], scalar1=NEG,
                                                        scalar2=None, op0=Alu.mult)
                                nc.vector.tensor_tensor(out=zlall[:], in0=zball[:], in1=zlall[:],
                                                        op=Alu.max)
                                ex_all = p3v.tile([128, TPG * 4], f32, tag="ex_all")
                                nc.scalar.activation(ex_all[:], zlall[:], Act.Exp)
                                ex16_all = p3v.tile([128, TPG * 4], bf16, tag="ex16_all")
                                nc.vector.tensor_copy(ex16_all[:], ex_all[:])
                                # loop B: weighted vals + segment matmuls
                                for k in range(TPG):
                                    vals = p3v.tile([128, HC], bf16, tag="vals")
                                    exap = ex16_all[:]
                                    exb = bass.AP(exap.tensor, exap.offset + k * 4,
                                                  [exap.ap[0], [1, 4], [0, cout]])
                                    nc.vector.tensor_tensor(
                                        out=vals[:].rearrange("p (h c) -> p h c", h=H),
                                        in0=gth[:, k, :HC].bitcast(bf16).rearrange("p (h c) -> p h c", h=H),
                                        in1=exb, op=Alu.mult)
                                    for q in range(HC // 512):
                                        nc.tensor.matmul(ps_agg[:, q * 512:(q + 1) * 512],
                                                         lhsT=M_all[:, k, :], rhs=vals[:, q * 512:(q + 1) * 512],
                                                         start=(k == 0), stop=(k == TPG - 1),
                                                         skip_group_check=True)
                                    nc.tensor.matmul(ps_den[:], lhsT=M_all[:, k, :],
                                                     rhs=ex16_all[:, k * 4:(k + 1) * 4],
                                                     start=(k == 0), stop=(k == TPG - 1),
                                                     skip_group_check=True)
                                # ---- finalize group: self-loop, normalize, mean heads ----
                                fin = p3s.tile([128, cout], f32, tag="fin")
                                zs = p3v.tile([128, 4], f32, tag="zs")
                                nc.vector.tensor_tensor(out=zs[:], in0=own_as, in1=own_ad, op=Alu.add)
                                nc.vector.tensor_tensor(out=zs[:], in0=zs[:],
                                                        in1=malE[:, gi, l * 4:(l + 1) * 4], op=Alu.add)
                                zs2 = p3v.tile([128, 4], f32, tag="zs2")
                                nc.vector.tensor_scalar(out=zs2[:], in0=zs[:], scalar1=NEG,
                                                        scalar2=None, op0=Alu.mult)
                                nc.vector.tensor_tensor(out=zs2[:], in0=zs[:], in1=zs2[:], op=Alu.max)
                                exs = p3v.tile([128, 4], f32, tag="exs")
                                nc.scalar.activation(exs[:], zs2[:], Act.Exp)
                                den = p3v.tile([128, 4], f32, tag="den")
                                nc.vector.tensor_tensor(out=den[:], in0=ps_den[:], in1=exs[:], op=Alu.add)
                                nc.vector.tensor_scalar(out=den[:], in0=den[:], scalar1=float(H),
                                                        scalar2=None, op0=Alu.mult)
                                rden = p3v.tile([128, 4], f32, tag="rden")
                                nc.vector.reciprocal(rden[:], den[:])
                                acc4 = p3v.tile([128, cout, 4], f32, tag="acc4")
                                t1 = p3v.tile([128, cout], f32, tag="t1")
                                C = cout
                                own_xs = ownrow[:, :HC].bitcast(bf16)
                                for h in range(H):
                                    nc.vector.tensor_scalar(out=t1[:, :C], in0=own_xs[:, h * C:(h + 1) * C],
                                                            scalar1=exs[:, h:h + 1], scalar2=None, op0=Alu.mult)
                                    nc.vector.tensor_tensor(out=t1[:, :C], in0=t1[:, :C],
                                                            in1=ps_agg[:, h * C:(h + 1) * C], op=Alu.add)
                                    nc.vector.tensor_scalar(out=acc4[:, :, h], in0=t1[:, :C],
                                                            scalar1=rden[:, h:h + 1], scalar2=None, op0=Alu.mult)
                                nc.vector.tensor_reduce(out=fin[:], in_=acc4[:],
                                                        axis=mybir.AxisListType.X, op=Alu.add)
                                nc.vector.tensor_tensor(out=fin[:], in0=fin[:], in1=bb[:], op=Alu.add)
                                # elu = max(x,0)-1 + exp(min(x,0))
                                en = p3v.tile([128, cout], f32, tag="en")
                                nc.vector.tensor_scalar(out=en[:], in0=fin[:], scalar1=0.0,
                                                        scalar2=None, op0=Alu.min)
                                ee = p3v.tile([128, cout], f32, tag="ee")
                                nc.scalar.activation(ee[:], en[:], Act.Exp)
                                nc.vector.tensor_scalar(out=fin[:], in0=fin[:], scalar1=0.0,
                                                        scalar2=-1.0, op0=Alu.max, op1=Alu.add)
                                nc.vector.tensor_tensor(out=fin[:], in0=fin[:], in1=ee[:], op=Alu.add)
                                if l < 2:
                                    fin16 = p3v.tile([128, cout], bf16, tag="fin16")
                                    nc.vector.tensor_copy(fin16[:], fin[:])
                                    for q in range(cout // 128):
                                        ps_tp = p3q.tile([128, 128], bf16, tag="pssm", space="PSUM")
                                        nc.tensor.transpose(ps_tp[:], fin16[:, q * 128:(q + 1) * 128], ident[:])
                                        nc.vector.tensor_copy(hT_acc[:, q, gi * 128:(gi + 1) * 128], ps_tp[:])
                                else:
                                    Mg = p3v.tile([128, G], bf16, tag="Mg")
                                    nc.vector.tensor_tensor(out=Mg[:], in0=bg_s[:, gi:gi + 1].to_broadcast([128, G]),
                                                            in1=iota64_f[:], op=Alu.is_equal)
                                    fin16 = p3v.tile([128, cout], bf16, tag="fin16")
                                    nc.vector.tensor_copy(fin16[:], fin[:])
                                    nc.tensor.matmul(ps_pool[:], lhsT=Mg[:], rhs=fin16[:],
                                                     start=(gi == 0), stop=(gi == NGRP - 1),
                                                     skip_group_check=True)
                            if l == 2:
                                pls = p3s.tile([G, 128], f32, tag="pls")
                                nc.vector.tensor_copy(pls[:], ps_pool[:])
                                nc.gpsimd.dma_start(pool_in[:], pls[:])

                    if l < 2:
                        for q in range(cout // 128):
                            nc.sync.dma_start(hTs[l][q * 128:(q + 1) * 128, :], hT_acc[:, q, :])
                        nc.gpsimd.collective_compute(
                            "AllGather", mybir.AluOpType.bypass,
                            replica_groups=[list(range(NCORES))],
                            ins=[hTs[l][:].opt()], outs=[hTall[l][:].opt()])

        nc.gpsimd.collective_compute(
            "AllReduce", mybir.AluOpType.add,
            replica_groups=[list(range(NCORES))],
            ins=[pool_in[:].opt()], outs=[pool_out[:].opt()])
        with tc.tile_pool(name="fino", bufs=1) as fo:
            pfin = fo.tile([G, 128], f32, name="pfin")
            nc.sync.dma_start(pfin[:], pool_out[:])
            nc.vector.tensor_scalar(out=pfin[:], in0=pfin[:], scalar1=ivc_s[:],
                                    scalar2=None, op0=Alu.mult)
            nc.sync.dma_start(t_out[:], pfin[:])
        res_ctx.__exit__(None, None, None)
        dram_sh_ctx.__exit__(None, None, None)

    nc.compile()
    return nc


def _null():
    from contextlib import nullcontext
    return nullcontext(None)


def kernel(**inputs):
    from concourse.bass_utils import run_bass_kernel_spmd

    p = _prep(inputs["x"], inputs["edge_index"], inputs["edge_attr"], inputs["batch"])
    TPG = p["TPG"]
    if TPG not in _cache:
        _cache[TPG] = _build(TPG)
    nc = _cache[TPG]

    base = {"xT": p["xT"], "invcnt": p["invcnt"]}
    for l in range(3):
        HC = H * DIMS[l][1]
        base[f"W{l+1}"] = np.ascontiguousarray(inputs[f"W{l+1}"], dtype=np.float32)
        base[f"We{l+1}"] = np.ascontiguousarray(inputs[f"We{l+1}"], dtype=np.float32)
        base[f"acat{l+1}"] = np.concatenate(
            [np.asarray(inputs[f"as{l+1}"], np.float32).reshape(1, HC),
             np.asarray(inputs[f"ad{l+1}"], np.float32).reshape(1, HC),
             np.asarray(inputs[f"ae{l+1}"], np.float32).reshape(1, HC)], axis=1)
        base[f"b{l+1}"] = np.asarray(inputs[f"b{l+1}"], np.float32).reshape(1, -1)
    in_maps = []
    for c in range(NCORES):
        m = dict(base)
        m["srcidx"] = p["srcidx"][c]
        m["dstf"] = p["dstf"][c]
        m["eattrT"] = p["eattrT"][c]
        m["invdeg"] = p["invdeg"][c]
        m["batchg"] = p["batchg"][c]
        in_maps.append(m)

    res = run_bass_kernel_spmd(nc, in_maps, core_ids=list(range(NCORES)))
    return np.asarray(res.results[0]["out"], dtype=np.float32)

